# revision 85
# baseline (speedup 1.0000x reference)
"""Multi-head attention (softmax over the QUERY axis) for Trainium2, 8 cores.

Reference (B=2, T=2048, E=1024, H=16, HD=64):
    q = split_heads(X @ Wq.T + bq); k, v likewise
    scores = (q @ k^T) / sqrt(E), causal (key > query -> -inf)
    attn   = softmax(scores, axis=QUERY)   # normalizes over q, per key
    out    = attn @ v

Sharding: core c = batch c//4, head group c%4 (4 heads = 256 dims).  No
collectives.  Host pre-transposes/casts so the device never transposes.

Per-core pipeline (ACT exp stream is the bottleneck engine; everything else
is scheduled around keeping it saturated):
  - X and W in fp8e4m3 (W prescaled x8 on host); Q/K/V projections run as
    DoubleRow fp8 matmuls (0.5 cyc/row): lhsT/rhs carry two 128-row
    e-slabs in the free dim, quartering projection PE time vs fp32r.
  - Q^T/K^T [128d, 2048t] fp16 in SBUF; scores S^T[k, q] per 128-key tile
    computed 512 cols at a time into fp32 PSUM; causal diag-block mask is
    ADDED ON THE PE (maskT @ I accumulate) instead of a DVE pass.
  - exp via ACT in <=1536-wide pieces (PSUM slots 2x[128,1536] = 6 banks),
    one P tile per (k-tile-pair, head) in fp8e4m3.  Final piece of each
    row uses accum_out for the per-key rowsum c[k]; non-final pieces are
    summed on DVE instead (saves the 187ns ACT accumulator read).
    rinv = 1/c on DVE; V'' = (v+bv)/c in fp8e5m2.
  - A@V: per head, k-tiles processed in PAIRS as DoubleRow fp8 matmuls
    (slabs = the two k-tiles), halving A@V PE time; each pair's 128-wide
    diagonal strip is a plain fp8 matmul.  O^T accumulates bank-major into
    per-head [64, 512] PSUM accumulators (DoubleRow requires dst partition
    0) while P tiles stay resident in SBUF; copied out + DMA'd per bank.
  - Softmax-over-query concentrates attention at late queries (c[k] ~ 1 at
    the causal-triangle tip), where quantization error cannot average out:
    the last k-tile pair runs fully in fp16, including its V projection
    from an fp16 copy of X's last t-block.
  - Scheduling: PE clock-ramp warm-up during input DMA; X DMA'd in query
    order with a column-major wavefront over duo-0 pairs 0-1 so exp starts
    ~6us in; short-piece-first splits hide PSUM slot turnaround; A@V
    body/close parts are slotted between score rows; the tail's last copy
    runs on the then-idle ACT engine.
"""

from collections import deque
from contextlib import ExitStack

import ml_dtypes
import numpy as np

import concourse.bacc as bacc
import concourse.mybir as mybir
import concourse.tile as tile
from concourse.bass_utils import run_bass_kernel_spmd

B, T, E, H = 2, 2048, 1024, 16
D2 = 256                 # output dims per core (4 heads)
F32 = mybir.dt.float32
F16 = mybir.dt.float16
F8E4 = mybir.dt.float8e4
F8E5 = mybir.dt.float8e5
EXP = mybir.ActivationFunctionType.Exp
DR = mybir.MatmulPerfMode.DoubleRow
MULT = mybir.AluOpType.mult
AX = mybir.AxisListType.X
WS = 8.0                        # host prescale on W/bias (fp8 range health)
SCALE = 1.0 / (32.0 * WS * WS)  # exp reads (8q).(8k) sums
MASKVAL = -60000.0              # fp16-representable; exp -> 0 after scale
SPLIT = 1536                    # PSUM score-slot width (3 banks)

_CACHE = {}


def _pieces(w):
    # Short piece first: the short exp then overlaps the long piece's PE
    # score production at PSUM-slot turnaround.
    return [(0, w)] if w <= SPLIT else [(0, 512), (512, w - 512)]


def _build_module():
    nc = bacc.Bacc("TRN2", target_bir_lowering=False, debug=False)

    xt_d = nc.dram_tensor("xt", [128, 16384], F8E4, kind="ExternalInput")
    # [wq-duo0 | wk-duo0 | wq-duo1 | wk-duo1], 1024 cols each: the duo-0
    # halves arrive as ONE DMA, shortening the first-score critical path.
    wqk_d = nc.dram_tensor("wqk", [128, 4096], F8E4, kind="ExternalInput")
    wv_d = nc.dram_tensor("wv", [128, 2048], F8E4, kind="ExternalInput")
    mi_d = nc.dram_tensor("mi", [128, 256], F16, kind="ExternalInput")
    bb_d = nc.dram_tensor("bb", [128, 4], F32, kind="ExternalInput")
    ob_d = nc.dram_tensor("ob", [1, 384], F16, kind="ExternalInput")
    x16_d = nc.dram_tensor("x16", [128, 2048], F16, kind="ExternalInput")
    wv16_d = nc.dram_tensor("wv16", [128, 2048], F16, kind="ExternalInput")
    # Output layout [dp, duo*2+hh, q]: per-head [64,512] banks DMA with the
    # same efficiency, and the final two heads merge into ONE tail DMA.
    ot_d = nc.dram_tensor("ot", [64, 4, T], F32, kind="ExternalOutput")

    with tile.TileContext(nc) as tc:
        _body(tc, xt_d, wqk_d, wv_d, mi_d, bb_d, ob_d, x16_d, wv16_d,
              ot_d)
    nc.compile()
    return nc


def _body(tc, xt_d, wqk_d, wv_d, mi_d, bb_d, ob_d, x16_d, wv16_d, ot_d):
    nc = tc.nc

    with ExitStack() as ctx:
        pool = {}
        for name, bufs, space in (
            ("const", 1, None), ("xw", 1, None), ("qk", 1, None),
            ("vsb", 1, None), ("p3", 20, None), ("vp", 20, None),
            ("st", 3, None), ("osb", 2, None),
            ("sc", 2, "PSUM"), ("otv", 1, "PSUM"),
        ):
            kw = {"name": name, "bufs": bufs}
            if space:
                kw["space"] = space
            pool[name] = ctx.enter_context(tc.tile_pool(**kw))

        mi_t = pool["const"].tile([128, 256], F16)       # maskT | identity
        bb_t = pool["const"].tile([128, 4], F32)         # 8bq (2 duos) | 8bk
        ob_t = pool["const"].tile([1, 384], F16)         # 8bv | ones
        warm_t = pool["const"].tile([1, 2], F32)
        wmm_t = pool["const"].tile([1, 512], F16)        # PE clock-ramp fuel
        xt_t = pool["xw"].tile([128, 64, 256], F8E4)     # [p, tb*8+ec, ti]
        wqk0_t = pool["xw"].tile([128, 16, 128], F8E4)  # wq0 | wk0
        wq1_t = pool["xw"].tile([128, 8, 128], F8E4)
        wk1_t = pool["xw"].tile([128, 8, 128], F8E4)
        wv_t = pool["xw"].tile([128, 8, 256], F8E4)
        x16_t = pool["xw"].tile([128, 8, 256], F16)      # X t-block 7, fp16
        wv16_t = pool["xw"].tile([128, 8, 256], F16)
        qtt = [pool["qk"].tile([128, T], F16, name=f"qt{d}") for d in (0, 1)]
        ktt = [pool["qk"].tile([128, T], F16, name=f"kt{d}") for d in (0, 1)]
        v_t = pool["vsb"].tile([128, 16 * D2], F16)      # 8(v+bv), [p, kt*256+d]

        # Warm the exp table off the critical path (no data deps).
        nc.vector.memset(warm_t[:, :], 0.0)
        nc.scalar.activation(warm_t[:, :], warm_t[:, :], EXP, bias=0.0,
                             scale=1.0)

        # PE clock-ramp warm-up: dummy matmuls with no DMA deps keep the PE
        # continuously busy through its ~3us p-state ramp while inputs
        # stream in, so the first real projections run at full clock.
        nc.gpsimd.memset(wmm_t[:, :], 0.0)
        warm_sc = pool["sc"].tile([128, SPLIT], F32, tag="sc", name="warm_sc")
        for _ in range(7):
            nc.tensor.matmul(warm_sc[:, 0:512], lhsT=wmm_t[0:1, 0:128],
                             rhs=wmm_t[0:1, 0:512], start=True, stop=True)

        # DMAs, first-use order (xt tb arrives in query order; the duo-0
        # pair-0/1 wavefront consumes it as it lands).
        def dma_xt(tb):
            nc.sync.dma_start(xt_t[:, 8 * tb:8 * tb + 8, :],
                              xt_d.ap()[:, 2048 * tb:2048 * tb + 2048])

        nc.sync.dma_start(wqk0_t[:, :, :], wqk_d.ap()[:, 0:2048])
        dma_xt(0)
        nc.sync.dma_start(bb_t[:, :], bb_d.ap())
        nc.sync.dma_start(mi_t[:, :], mi_d.ap())
        dma_xt(1)
        dma_xt(2)
        dma_xt(3)
        dma_xt(4)
        dma_xt(5)
        dma_xt(6)
        dma_xt(7)
        nc.sync.dma_start(wv_t[:, :, :], wv_d.ap())
        nc.sync.dma_start(ob_t[:, :], ob_d.ap())
        nc.sync.dma_start(wq1_t[:, :, :], wqk_d.ap()[:, 2048:3072])
        nc.sync.dma_start(wk1_t[:, :, :], wqk_d.ap()[:, 3072:4096])
        nc.sync.dma_start(x16_t[:, :, :], x16_d.ap())
        nc.sync.dma_start(wv16_t[:, :, :], wv16_d.ap())

        pair_store = {}
        av_state = {}
        ovflip = [0]

        def ov_tag():
            # Proj/V chunks ping-pong across the two 1-bank psum slots that
            # the per-head A@V accumulators also use.
            ovflip[0] ^= 1
            return "ova" if ovflip[0] else "ovb"

        def emit_qk_chunk(is_k, duo, u, subs=(0, 1)):
            # [128d, 256|512 t] projection chunk for Q^T/K^T of `duo`.
            wide = 256 * len(subs)
            ps = pool["otv"].tile([128, wide], F32, tag=ov_tag(),
                                  name="ps_qk")
            if duo == 0:
                w_t = wqk0_t[:, 8 * int(is_k):8 * int(is_k) + 8, :]
            else:
                w_t = (wk1_t if is_k else wq1_t)[:, :, :]
            out_sb = ktt[duo] if is_k else qtt[duo]
            for si, sub in enumerate(subs):
                tb = 2 * u + sub
                for c in range(4):
                    nc.tensor.matmul(
                        ps[:, 256 * si:256 * si + 256],
                        lhsT=w_t[:, 2 * c:2 * c + 2, :],
                        rhs=xt_t[:, 8 * tb + 2 * c:8 * tb + 2 * c + 2, :],
                        start=(si == 0 and c == 0),
                        stop=(si == len(subs) - 1 and c == 3),
                        perf_mode=DR)
            col = 2 * int(is_k) + duo
            c0 = 512 * u + 256 * subs[0]
            nc.vector.tensor_scalar_add(
                out_sb[:, c0:c0 + wide], ps[:, :], bb_t[:, col:col + 1])

        def emit_v_pair(tp):
            # V tiles tt=2tp,2tp+1: [128t, 256d] each, + bias, -> v_t (fp16).
            # The last pair (keys 1792..2047) runs in fp16: those keys feed
            # the concentrated softmax tip, where projection error is
            # uncancelled.
            ps = pool["otv"].tile([128, 512], F32, tag=ov_tag(), name="ps_v")
            first = True
            for sub in (0, 1):
                toff = 128 * sub
                for c in range(4):
                    if tp == 7:
                        for cc in (2 * c, 2 * c + 1):
                            nc.tensor.matmul(
                                ps[:, 256 * sub:256 * sub + 256],
                                lhsT=x16_t[:, cc:cc + 1, toff:toff + 128],
                                rhs=wv16_t[:, cc:cc + 1, :],
                                start=first, stop=False)
                            first = False
                    else:
                        nc.tensor.matmul(
                            ps[:, 256 * sub:256 * sub + 256],
                            lhsT=xt_t[:, 8 * tp + 2 * c:8 * tp + 2 * c + 2,
                                      toff:toff + 128],
                            rhs=wv_t[:, 2 * c:2 * c + 2, :],
                            start=first, stop=False, perf_mode=DR)
                        first = False
                nc.tensor.matmul(
                    ps[:, 256 * sub:256 * sub + 256],
                    lhsT=ob_t[0:1, 256:384], rhs=ob_t[0:1, 0:256],
                    start=False, stop=(sub == 1))
            nc.vector.tensor_copy(v_t[:, 512 * tp:512 * tp + 512], ps[:, :])

        def emit_av(duo, b, js, close):
            # O^T bank b (queries [512b, 512b+512)) += sum over k-tile pairs.
            st = av_state.get((duo, b))
            if st is None:
                st = {"ot": [pool["otv"].tile([64, 512], F32,
                                              tag=("ova", "ovb")[hh],
                                              name=f"av{duo}{b}h{hh}")
                             for hh in (0, 1)],
                      "started": [False, False]}
                av_state[(duo, b)] = st
            q0 = 512 * b
            q1 = q0 + 512
            ms = []
            for j in js:
                qlo_a = 256 * j
                if j >= 2 * b:
                    ms.append(("strip", j, qlo_a, qlo_a, 128))
                c0 = max(q0, qlo_a + 128)
                while c0 < q1:
                    n = min(256, q1 - c0)
                    ms.append(("dr", j, qlo_a, c0, n))
                    c0 += n
            for hh in (0, 1):
                ot = st["ot"][hh]
                for idx, (kind, j, qlo_a, c0, n) in enumerate(ms):
                    start = not st["started"][hh]
                    st["started"][hh] = True
                    stop = close and (idx == len(ms) - 1)
                    p3_t, vp_t = pair_store[(duo, j)][hh]
                    if kind == "strip":
                        nc.tensor.matmul(
                            ot[:, c0 - q0:c0 - q0 + n],
                            lhsT=vp_t[:, 0:1, :],
                            rhs=p3_t[:, 0:1, 0:n],
                            start=start, stop=stop)
                    elif j == 7:
                        # fp16 pair: one plain matmul per k-tile slab.
                        nc.tensor.matmul(
                            ot[:, c0 - q0:c0 - q0 + n],
                            lhsT=vp_t[:, 0:1, :],
                            rhs=p3_t[:, 0:1, c0 - qlo_a:c0 - qlo_a + n],
                            start=start, stop=False)
                        nc.tensor.matmul(
                            ot[:, c0 - q0:c0 - q0 + n],
                            lhsT=vp_t[:, 1:2, :],
                            rhs=p3_t[:, 1:2, c0 - qlo_a:c0 - qlo_a + n],
                            start=False, stop=stop)
                    else:
                        nc.tensor.matmul(
                            ot[:, c0 - q0:c0 - q0 + n],
                            lhsT=vp_t[:, 0:2, :],
                            rhs=p3_t[:, 0:2, c0 - qlo_a:c0 - qlo_a + n],
                            start=start, stop=stop, perf_mode=DR)
                if close:
                    dh = 2 * duo + hh
                    if duo == 1 and b == 3:
                        # Kernel tail: both heads land in one [64,2,512]
                        # staging tile (head 1 copied by the then-idle ACT
                        # engine) and ship as a single DMA.
                        if hh == 0:
                            o3 = pool["osb"].tile([64, 2, 512], F32,
                                                  tag="osb", name="o3")
                            st["o3"] = o3
                            nc.vector.tensor_copy(o3[:, 0:1, :], ot[:, :])
                        else:
                            o3 = st["o3"]
                            nc.scalar.copy(o3[:, 1:2, :], ot[:, :])
                            nc.sync.dma_start(
                                ot_d.ap()[0:64, 2:4, q0:q1], o3[:, :, :])
                    else:
                        o_t = pool["osb"].tile([64, 512], F32, tag="osb",
                                               name="osb")
                        nc.vector.tensor_copy(o_t[:, :], ot[:, :])
                        nc.sync.dma_start(
                            ot_d.ap()[0:64, dh:dh + 1, q0:q1], o_t[:, :])

        tiles = {}

        def make_pair_tiles(duo, j):
            wa = T - 256 * j
            pdt, vdt = (F16, F16) if j == 7 else (F8E4, F8E5)
            pair_store[(duo, j)] = [
                (pool["p3"].tile([128, 2, wa], pdt, tag="p3", name="p3"),
                 pool["vp"].tile([128, 2, 64], vdt, tag="vp", name="vp"))
                for _ in (0, 1)]
            sums = pool["st"].tile([128, 16], F32, tag="sums", name="sums")
            rsum = pool["st"].tile([128, 4], F32, tag="rsum", name="rsum")
            rinv = pool["st"].tile([128, 4], F32, tag="rinv", name="rinv")
            tiles[(duo, j)] = (sums, rsum, rinv)

        def emit_row_piece(duo, j, s, hh, poff, pw, pi, npieces,
                           defer_sums=None, borrow=False):
            # One exp piece of row (k-tile 2j+s, head hh): scores into a
            # PSUM slot, diag mask on the PE, exp+rowsum on ACT.  During
            # the wavefront the idle A@V psum banks serve as extra score
            # slots (borrow=True, pieces <= 512 wide) to widen the
            # exp-pipeline while X still streams in.
            p3_t, _ = pair_store[(duo, j)][hh]
            sums, rsum, _ = tiles[(duo, j)]
            kti = 2 * j + s
            qlo = 128 * kti
            if borrow:
                sc = pool["otv"].tile([128, 512], F32, tag=ov_tag(),
                                      name="sc_ov")
            else:
                sc = pool["sc"].tile([128, SPLIT], F32, tag="sc", name="sc")
            for co in range(0, pw, 512):
                n = min(512, pw - co)
                nc.tensor.matmul(
                    sc[:, co:co + n],
                    lhsT=ktt[duo][64 * hh:64 * hh + 64, qlo:qlo + 128],
                    rhs=qtt[duo][64 * hh:64 * hh + 64,
                                 qlo + poff + co:qlo + poff + co + n],
                    start=True,
                    stop=not (poff == 0 and co == 0))
            if poff == 0:
                nc.tensor.matmul(
                    sc[:, 0:128], lhsT=mi_t[:, 0:128],
                    rhs=mi_t[:, 128:256], start=False, stop=True)
            ci = 2 * s + hh
            p3_sl = p3_t[:, s:s + 1, 128 * s + poff:128 * s + poff + pw]
            if pi < npieces - 1:
                # Non-final piece: skip the ACT accumulator read (187 ns on
                # the bottleneck engine); DVE sums the quantized P instead.
                # The reduce may be deferred so it doesn't head-block other
                # DVE work (the queue is in-order).
                nc.scalar.activation(p3_sl, sc[:, 0:pw], EXP, bias=0.0,
                                     scale=SCALE)
                red = lambda: nc.vector.reduce_sum(
                    sums[:, 4 * ci + pi:4 * ci + pi + 1], p3_sl, axis=AX)
                if defer_sums is None:
                    red()
                else:
                    defer_sums.append(red)
            else:
                acc = (rsum[:, ci:ci + 1] if npieces == 1
                       else sums[:, 4 * ci + pi:4 * ci + pi + 1])
                nc.scalar.activation(p3_sl, sc[:, 0:pw], EXP, bias=0.0,
                                     scale=SCALE, accum_out=acc)

        def finish_slab(duo, j, s, nps):
            # Combine partial rowsums, rinv, V'' for slab s (k-tile 2j+s).
            sums, rsum, rinv = tiles[(duo, j)]
            for hh in (0, 1):
                n = nps[(s, hh)]
                if n > 1:
                    ci = 2 * s + hh
                    nc.vector.tensor_add(
                        rsum[:, ci:ci + 1],
                        sums[:, 4 * ci:4 * ci + 1],
                        sums[:, 4 * ci + 1:4 * ci + 2])
                    for pi in range(2, n):
                        nc.vector.tensor_add(
                            rsum[:, ci:ci + 1],
                            rsum[:, ci:ci + 1],
                            sums[:, 4 * ci + pi:4 * ci + pi + 1])
            nc.vector.reciprocal(rinv[:, 2 * s:2 * s + 2],
                                 rsum[:, 2 * s:2 * s + 2])
            for hh in (0, 1):
                _, vp_t = pair_store[(duo, j)][hh]
                c0 = D2 * (2 * j + s) + 128 * duo + 64 * hh
                nc.vector.tensor_scalar(
                    vp_t[:, s:s + 1, :],
                    v_t[:, c0:c0 + 64],
                    rinv[:, 2 * s + hh:2 * s + hh + 1],
                    1.0 / WS, MULT, MULT)

        def finish_pair(duo, j, nps):
            finish_slab(duo, j, 0, nps)
            finish_slab(duo, j, 1, nps)

        def emit_pair(duo, j, row_hooks, after):
            make_pair_tiles(duo, j)
            hooks = deque(row_hooks)
            nps = {}
            for s in (0, 1):
                pcs = _pieces(T - 128 * (2 * j + s))
                for hh in (0, 1):
                    nps[(s, hh)] = len(pcs)
                    for pi, (poff, pw) in enumerate(pcs):
                        emit_row_piece(duo, j, s, hh, poff, pw, pi, len(pcs))
                    if hooks:
                        f = hooks.popleft()
                        if f is not None:
                            f()
                finish_slab(duo, j, s, nps)
            for f in after:
                f()

        def emit_wavefront01():
            # Duo-0 pairs 0+1 (k-tiles 0..3), emitted column-major against
            # X's DMA arrival order so the exp stream starts while X still
            # streams in.  The very first row gets an extra 256-wide piece
            # so exp can begin on X t-block 0 alone.
            for j in (0, 1):
                make_pair_tiles(0, j)
            rows = [(j, s, hh) for j in (0, 1) for s in (0, 1)
                    for hh in (0, 1)]
            pcs = {}
            for j, s, hh in rows:
                qlo = 128 * (2 * j + s)
                if (j, s, hh) == (0, 0, 0):
                    # Extra splits: piece 0 starts on X t-block 0 alone;
                    # piece 2 needs only Q1/Q2, bridging the Q3 DMA chain.
                    pcs[(j, s, hh)] = [(0, 256), (256, 256), (512, 1024),
                                       (1536, 512)]
                else:
                    pcs[(j, s, hh)] = [(0, 512 - qlo), (512 - qlo, T - 512)]
            dsums = []
            emit_row_piece(0, 0, 0, 0, 0, 256, 0, 4, defer_sums=dsums)
            emit_qk_chunk(False, 0, 0, subs=(1,))
            emit_qk_chunk(True, 0, 0, subs=(1,))
            emit_row_piece(0, 0, 0, 0, 256, 256, 1, 4, defer_sums=dsums)
            for ri, r in enumerate(rows[1:]):
                j, s, hh = r
                poff, pw = pcs[r][0]
                emit_row_piece(0, j, s, hh, poff, pw, 0, len(pcs[r]),
                               defer_sums=dsums, borrow=(ri % 2 == 0))
                if ri == 1:
                    emit_qk_chunk(False, 0, 1)
                elif ri == 3:
                    emit_qk_chunk(False, 0, 2)
            emit_row_piece(0, 0, 0, 0, 512, 1024, 2, 4, defer_sums=dsums)
            emit_qk_chunk(False, 0, 3)
            for red in dsums:
                red()
            for r in rows:
                j, s, hh = r
                poff, pw = pcs[r][-1]
                emit_row_piece(0, j, s, hh, poff, pw, len(pcs[r]) - 1,
                               len(pcs[r]))
            emit_qk_chunk(True, 0, 1)
            for tp in range(8):
                emit_v_pair(tp)
            for j in (0, 1):
                finish_pair(0, j, {(s, hh): len(pcs[(j, s, hh)])
                                   for s in (0, 1) for hh in (0, 1)})

        # Upfront: only what the very first piece needs (X t-block 0).
        emit_qk_chunk(False, 0, 0, subs=(0,))
        emit_qk_chunk(True, 0, 0, subs=(0,))
        emit_wavefront01()

        # Row-slot schedule: r3 of each pair carries an A@V part; slots
        # between an A@V body and its close must stay free of otv-psum
        # users (proj chunks / v-pairs) so the accumulator tiles aren't
        # recycled mid-group.
        def qk2(a, b):
            def f():
                emit_qk_chunk(*a)
                emit_qk_chunk(*b)
            return f

        row_hooks = {
            (0, 2): [qk2((True, 0, 2), (True, 0, 3)),
                     lambda: emit_qk_chunk(False, 1, 0),
                     lambda: emit_av(0, 0, [0, 1], True),
                     lambda: emit_qk_chunk(False, 1, 1)],
            (0, 3): [lambda: emit_qk_chunk(False, 1, 2),
                     lambda: emit_qk_chunk(False, 1, 3),
                     lambda: emit_av(0, 1, [0, 1], False),
                     None],
            (0, 4): [None, None,
                     lambda: emit_av(0, 1, [2, 3], True),
                     qk2((True, 1, 0), (True, 1, 1))],
            (0, 5): [lambda: emit_qk_chunk(True, 1, 2),
                     lambda: emit_qk_chunk(True, 1, 3),
                     lambda: emit_av(0, 2, [0, 1, 2, 3], False),
                     None],
            (0, 6): [None, None,
                     lambda: emit_av(0, 2, [4, 5], True),
                     lambda: emit_av(0, 3, [0, 1, 2, 3, 4, 5], False)],
            (1, 0): [None, None,
                     lambda: emit_av(0, 3, [6, 7], True),
                     None],
            (1, 2): [None, None,
                     lambda: emit_av(1, 0, [0, 1], True),
                     None],
            (1, 3): [None, None,
                     lambda: emit_av(1, 1, [0, 1], False),
                     None],
            (1, 4): [None, None,
                     lambda: emit_av(1, 1, [2, 3], True),
                     None],
            (1, 5): [None, None,
                     lambda: emit_av(1, 2, [0, 1, 2, 3], False),
                     None],
            (1, 6): [None, None,
                     lambda: emit_av(1, 2, [4, 5], True),
                     lambda: emit_av(1, 3, [0, 1, 2, 3, 4, 5], False)],
        }
        after_hooks = {
            (1, 7): [lambda: emit_av(1, 3, [6, 7], True)],
        }
        for duo in (0, 1):
            for j in range(8):
                if duo == 0 and j < 2:
                    continue
                emit_pair(duo, j, row_hooks.get((duo, j), []),
                          after_hooks.get((duo, j), []))


def _get_module():
    if "nc" not in _CACHE:
        _CACHE["nc"] = _build_module()
    return _CACHE["nc"]


def _make_in_maps(X, Wq, bq, Wk, bk, Wv, bv):
    e4 = ml_dtypes.float8_e4m3
    X = np.asarray(X, np.float32)
    a = np.arange(128)
    mask_t = np.where(a[:, None] < a[None, :], np.float32(MASKVAL),
                      np.float32(0.0))
    mi = np.concatenate([mask_t, np.eye(128, dtype=np.float32)],
                        axis=1).astype(np.float16)
    in_maps = []
    for c in range(8):
        b, g = divmod(c, 4)
        rows = slice(D2 * g, D2 * g + D2)

        xt = X[b].T.reshape(8, 128, 8, 256).transpose(1, 2, 0, 3)

        def wprep(w, duo_major):
            wc = (np.asarray(w, np.float32)[rows] * WS).T
            if duo_major:     # [p, duo*1024 + ec*128 + d]
                wc = wc.reshape(8, 128, 2, 128).transpose(1, 2, 0, 3)
            else:             # [p, ec*256 + d]
                wc = wc.reshape(8, 128, D2).transpose(1, 0, 2)
            return np.ascontiguousarray(wc).reshape(128, 2048).astype(e4)

        wq2 = wprep(Wq, True)
        wk2 = wprep(Wk, True)
        x16 = X[b].T[:, 1792:2048].reshape(8, 128, 256).transpose(1, 0, 2)
        wv16 = (np.asarray(Wv, np.float32)[rows] * WS).T.reshape(
            8, 128, D2).transpose(1, 0, 2)
        bq8 = (np.asarray(bq, np.float32)[rows] * WS).reshape(2, 128).T
        bk8 = (np.asarray(bk, np.float32)[rows] * WS).reshape(2, 128).T
        bb = np.ascontiguousarray(
            np.concatenate([bq8, bk8], axis=1)).astype(np.float32)
        ob = np.zeros((1, 384), np.float32)
        ob[0, 0:D2] = np.asarray(bv, np.float32)[rows] * WS
        ob[0, D2:384] = 1.0
        in_maps.append({
            "xt": np.ascontiguousarray(xt).reshape(128, 16384).astype(e4),
            "wqk": np.ascontiguousarray(np.concatenate(
                [wq2[:, 0:1024], wk2[:, 0:1024],
                 wq2[:, 1024:2048], wk2[:, 1024:2048]], axis=1)),
            "wv": wprep(Wv, False),
            "x16": np.ascontiguousarray(x16).reshape(
                128, 2048).astype(np.float16),
            "wv16": np.ascontiguousarray(wv16).reshape(
                128, 2048).astype(np.float16),
            "mi": mi, "bb": bb, "ob": ob.astype(np.float16),
        })
    return in_maps


def kernel(X, Wq, bq, Wk, bk, Wv, bv, **kw):
    in_maps = _make_in_maps(X, Wq, bq, Wk, bk, Wv, bv)
    nc = _get_module()
    res = run_bass_kernel_spmd(nc, in_maps, core_ids=list(range(8)), **kw)
    _CACHE["last_res"] = res
    out = np.zeros((B, T, E), np.float32)
    for c in range(8):
        b, g = divmod(c, 4)
        o3 = np.asarray(res.results[c]["ot"])          # [64, 4, 2048]
        ot = o3.transpose(1, 0, 2).reshape(D2, T)      # [duo*128+hh*64+p, q]
        out[b, :, D2 * g:D2 * g + D2] = ot.T
    return out


if __name__ == "__main__":
    _get_module()
    print("module built ok")
